# revision 3
# baseline (speedup 1.0000x reference)
"""Trainium2 Bass kernel v2 for nn_HGraphAttentionLayer (8 NeuronCores).

Reference math:
  feats[h,n,o]  = concat(input[:5000] @ proj_rna[h], input[5000:] @ proj_dis[h])
  s_src[h,n]    = feats[h,n,:] @ score_src[h];  s_tgt likewise
  e[h,j,i]      = exp(lrelu(s_src[h,i] + s_tgt[h,j] + M[i,j], 0.2))
                  (M additive {0,-1e9}; lrelu(-1e9) -> exp -> exact 0)
  d[h,j]        = sum_i e[h,j,i]   (softmax denominator, global over i)
  vals[i,o]     = mean_h( sum_j (feats[h,j,o]/d[h,j]) * e[h,j,i] )
  out           = elu( instancenorm(vals) + input @ residual_w.T )

Per-tile chain ([128 j, 1024 i] bf16), engine-balanced:
  zm = M'_jt + S_h                    DVE tensor_add            (~0.69us)
  y  = lrelu(zm + t_j)                ACT Prelu(bias=t_j)       (~1.13us)
       or on DVE: q1 = zm + t_j; q2 = 0.2*zm + 0.2*t_j; y = max(q1,q2)
  e  = Exp(y) with accum -> d         ACT                       (~1.41us)

Sharding: row sharding; core k owns target rows i in [k*1024,(k+1)*1024).
e is laid out [j_partitions, i_free]; the mask ships from host pre-transposed
(just a dtype/layout transform). d partials AllGather per half-stripe
(4 j-chunks x 4 heads), pipelined against the next half-stripe's elementwise.
feats are computed just-in-time per half-stripe; nothing spills to DRAM.
"""
import numpy as np

N, F, H, O = 8192, 256, 4, 128
N_CORES = 8
MY_N = N // N_CORES          # 1024 rows per core
N_RNA = 5000
SLOPE = 0.2
EPS = 1e-5
NCH = N // 128               # 64 j-chunks
FC = F // 128                # 2 f-chunks
SPLIT_CH = N_RNA // 128      # chunk 39 contains the rna/dis boundary
SPLIT_ROW = N_RNA - SPLIT_CH * 128  # row 8 within chunk 39
N_HS = 16                    # half-stripes
CPH = NCH // N_HS            # 4 chunks per half-stripe

_cached = {}


def _build():
    import concourse.bass as bass
    import concourse.bacc as bacc
    import concourse.mybir as mybir
    import concourse.tile as tile

    f32 = mybir.dt.float32
    bf16 = mybir.dt.bfloat16
    Alu = mybir.AluOpType
    Act = mybir.ActivationFunctionType

    nc = bacc.Bacc("TRN2", target_bir_lowering=False, debug=False,
                   enable_asserts=False, num_devices=N_CORES)

    # ---- I/O -----------------------------------------------------------
    maskT_in = nc.dram_tensor("maskT", [N, MY_N], bf16, kind="ExternalInput").ap()
    inputT_in = nc.dram_tensor("inputT", [FC, 128, N], bf16,
                               kind="ExternalInput").ap()
    # own-row inputT [FC, 128, MY_N] and the rna-membership row for own rows
    inputT_my = nc.dram_tensor("inputT_my", [FC, 128, MY_N], bf16,
                               kind="ExternalInput").ap()
    rowmask_in = nc.dram_tensor("rowmask", [1, MY_N], bf16,
                                kind="ExternalInput").ap()
    proj_rna = nc.dram_tensor("proj_rna", [H, F, O], f32, kind="ExternalInput").ap()
    proj_dis = nc.dram_tensor("proj_dis", [H, F, O], f32, kind="ExternalInput").ap()
    score_src = nc.dram_tensor("score_src", [H, O, 1], f32, kind="ExternalInput").ap()
    score_tgt = nc.dram_tensor("score_tgt", [H, O, 1], f32, kind="ExternalInput").ap()
    residual_wT = nc.dram_tensor("residual_wT", [FC, 128, O], bf16,
                                 kind="ExternalInput").ap()
    identf_in = nc.dram_tensor("identf", [128, 128], f32, kind="ExternalInput").ap()
    sel39_in = nc.dram_tensor("sel39", [128, 1], f32, kind="ExternalInput").ap()
    invsel39_in = nc.dram_tensor("invsel39", [128, 1], f32, kind="ExternalInput").ap()
    out_dram = nc.dram_tensor("out", [O, MY_N], f32, kind="ExternalOutput").ap()

    RG = [list(range(N_CORES))]

    with tile.TileContext(nc) as tc:
        with (
            tc.tile_pool(name="const", bufs=1) as constp,
            tc.tile_pool(name="pro", bufs=3) as pro,
            tc.tile_pool(name="ps_work", bufs=1, space="PSUM") as ps_work,
            tc.tile_pool(name="ps_s", bufs=2, space="PSUM") as ps_s,
            tc.tile_pool(name="ps_feats", bufs=3, space="PSUM") as ps_feats,
            tc.tile_pool(name="ps_vals", bufs=1, space="PSUM") as ps_vals,
            tc.tile_pool(name="dram", bufs=1, space="DRAM") as dram,
        ):
            # ---- DRAM scratch ------------------------------------------
            d_in = [dram.tile([128, 16], f32, tag=f"din{s}", name=f"din{s}")
                    for s in range(N_HS)]
            d_out = [dram.tile([128 * N_CORES, 16], f32, tag=f"dout{s}",
                               name=f"dout{s}")
                     for s in range(N_HS)]
            d15_in = [dram.tile([128, 8], f32, tag=f"d15i{x}", name=f"d15i{x}")
                      for x in range(2)]
            d15_out = [dram.tile([128 * N_CORES, 8], f32, tag=f"d15o{x}",
                                 name=f"d15o{x}")
                       for x in range(2)]
            arow_dram = dram.tile([H, MY_N], bf16, tag="arowd", name="arowd")
            st_in = dram.tile([1, 32], f32, tag="stin", name="stin")
            st_out = dram.tile([1, 32], f32, tag="stout", name="stout")
            dum_in = dram.tile([1, 16], f32, tag="dumin", name="dumin")
            dum_out = dram.tile([1, 16], f32, tag="dumout", name="dumout")

            # ---- constants ---------------------------------------------
            identf = constp.tile([128, 128], f32, tag="identf", name="identf")
            nc.sync.dma_start(identf[:], identf_in)
            ones_col = constp.tile([128, 1], f32, tag="ones_col", name="ones_col")
            nc.vector.memset(ones_col[:], 1.0)
            ones_row = constp.tile([1, 512], f32, tag="ones_row", name="ones_row")
            nc.vector.memset(ones_row[:], 1.0)
            sel39 = constp.tile([128, 1], f32, tag="sel39", name="sel39")
            nc.sync.dma_start(sel39[:], sel39_in)
            invsel39 = constp.tile([128, 1], f32, tag="invsel39", name="invsel39")
            nc.sync.dma_start(invsel39[:], invsel39_in)

            # warm up the collective stack early
            zr = constp.tile([1, 16], f32, tag="zr", name="zr")
            nc.vector.memset(zr[:], 0.0)
            nc.sync.dma_start(dum_in[:], zr[:])
            nc.gpsimd.collective_compute(
                "AllReduce", Alu.add, replica_groups=RG,
                ins=[dum_in.opt()], outs=[dum_out.opt()])

            # ---- projections -> bf16, 4 heads side by side -------------
            # projb_all[(t, fc)] = [128 f, 4h*128 o] so feats is one FD=512 mm
            projb_all = {}
            for tname, pap in (("rna", proj_rna), ("dis", proj_dis)):
                for fc in range(FC):
                    pb = constp.tile([128, H * O], bf16, tag=f"pb_{tname}{fc}",
                                     name=f"pb_{tname}{fc}")
                    for h in range(H):
                        praw = pro.tile([128, O], f32, tag="praw", name="praw",
                                        bufs=2)
                        nc.sync.dma_start(praw[:], pap[h, fc * 128:(fc + 1) * 128, :])
                        nc.vector.tensor_copy(pb[:, h * 128:(h + 1) * 128], praw[:])
                    projb_all[(tname, fc)] = pb

            # ---- residual weight (host pre-transposed) -----------------
            wrT = []
            for fc in range(FC):
                wt = constp.tile([128, O], bf16, tag=f"wrT{fc}", name=f"wrT{fc}")
                nc.sync.dma_start(wt[:], residual_wT[fc])
                wrT.append(wt)

            # ---- own-row inputT + rna/dis zero-split on device ---------
            rmb = constp.tile([128, MY_N], bf16, tag="rmb", name="rmb")
            nc.sync.dma_start(rmb[:], rowmask_in.partition_broadcast(128))
            rmbi = constp.tile([128, MY_N], bf16, tag="rmbi", name="rmbi")
            nc.vector.tensor_scalar(rmbi[:], rmb[:], -1.0, 1.0,
                                    op0=Alu.mult, op1=Alu.add)
            myrnaT, mydisT, rowsT = [], [], []
            for fc in range(FC):
                rt = constp.tile([128, MY_N], bf16, tag=f"rowsT{fc}",
                                 name=f"rowsT{fc}")
                nc.sync.dma_start(rt[:], inputT_my[fc])
                rowsT.append(rt)
                ra = constp.tile([128, MY_N], bf16, tag=f"myrna{fc}",
                                 name=f"myrna{fc}")
                nc.vector.tensor_mul(ra[:], rt[:], rmb[:])
                myrnaT.append(ra)
                di = constp.tile([128, MY_N], bf16, tag=f"mydis{fc}",
                                 name=f"mydis{fc}")
                nc.vector.tensor_mul(di[:], rt[:], rmbi[:])
                mydisT.append(di)

            # ---- score vectors -> q[type][fc] = [128f, 8] bf16 ---------
            # q[f] = sum_o proj[h][f,o] * score[h][o], via PE with projT
            # cols 0..3 = src head h, 4..7 = tgt head h
            q_rhs = {(t, fc): constp.tile([128, 8], bf16, tag=f"q{t}{fc}",
                                          name=f"q{t}{fc}")
                     for t in ("rna", "dis") for fc in range(FC)}
            scols = {}
            for si, sap in ((0, score_src), (1, score_tgt)):
                for h in range(H):
                    sc = pro.tile([128, 1], f32, tag="scols", name="scols", bufs=8)
                    nc.sync.dma_start(sc[:], sap[h])
                    scols[(si, h)] = sc
            for tname, pap in (("rna", proj_rna), ("dis", proj_dis)):
                for fc in range(FC):
                    psq = ps_s.tile([128, 48], f32, tag="pss", name="psq")
                    for h in range(H):
                        praw2 = pro.tile([128, O], f32, tag="praw2", name="praw2",
                                         bufs=2)
                        nc.sync.dma_start(praw2[:],
                                          pap[h, fc * 128:(fc + 1) * 128, :])
                        tpp = ps_work.tile([128, 128], f32, tag="tp", name="tpp")
                        nc.tensor.transpose(tpp[:], praw2[:], identf[:])
                        pT = pro.tile([128, 128], f32, tag="pT", name="pT", bufs=2)
                        nc.vector.tensor_copy(pT[:], tpp[:])
                        for si in range(2):
                            nc.tensor.matmul(
                                psq[:, si * 4 + h:si * 4 + h + 1], pT[:],
                                scols[(si, h)][:], start=True, stop=True)
                    nc.vector.tensor_copy(q_rhs[(tname, fc)][:], psq[:, 0:8])

            # ---- own-row s_src -> S_h bcast tiles ----------------------
            for ic in range(MY_N // 128):
                ps_sr = ps_s.tile([128, 48], f32, tag="pss", name="pssr")
                k = 0
                for tname, Tt in (("rna", myrnaT), ("dis", mydisT)):
                    for fc in range(FC):
                        nc.tensor.matmul(ps_sr[:, 0:8],
                                         Tt[fc][:, ic * 128:(ic + 1) * 128],
                                         q_rhs[(tname, fc)][:],
                                         start=(k == 0), stop=(k == 3))
                        k += 1
                srow = pro.tile([128, 8], f32, tag="srow", name="srow", bufs=2)
                nc.vector.tensor_copy(srow[:], ps_sr[:, 0:8])
                tps = ps_work.tile([128, 128], f32, tag="tp", name="tps")
                nc.tensor.transpose(tps[0:8, :], srow[:], identf[:])
                srT = pro.tile([8, 128], bf16, tag="srT8", name="srT8", bufs=2)
                nc.vector.tensor_copy(srT[:], tps[0:8, :])
                for h in range(H):
                    nc.sync.dma_start(arow_dram[h, ic * 128:(ic + 1) * 128],
                                      srT[h:h + 1, :])
            # broadcast raw s_src rows (head h) to [128, MY_N] bf16
            S_b = []
            for h in range(H):
                sb = constp.tile([128, MY_N], bf16, tag=f"Sb{h}", name=f"Sb{h}")
                nc.sync.dma_start(sb[:],
                                  arow_dram[h:h + 1, :].partition_broadcast(128))
                S_b.append(sb)

            def chunk_type(ch):
                if ch < SPLIT_CH:
                    return "rna"
                if ch > SPLIT_CH:
                    return "dis"
                return "both"

            # ================= main loop over half-stripes ===============
            loop = tc.alloc_tile_pool(name="loop", bufs=3)
            epool = tc.alloc_tile_pool(name="epool", bufs=34)
            vals_ps = ps_vals.tile([128, MY_N], f32, tag="big", name="vals")
            first_mm = [True]
            pendings = []

            for hs in range(N_HS):
                chunks = [hs * CPH + c for c in range(CPH)]
                # -- mask tile prefetch (issue before compute work) ------
                mTs = []
                for c, ch in enumerate(chunks):
                    mT = loop.tile([128, MY_N], bf16, tag="mT", name="mT", bufs=12)
                    nc.sync.dma_start(mT[:], maskT_in[ch * 128:(ch + 1) * 128, :])
                    mTs.append(mT)
                # -- inputT chunk loads + s + feats (JIT) ----------------
                pss = ps_s.tile([128, 48], f32, tag="pss", name=f"pss{hs}")
                fsb = []   # per chunk [128, H*128] bf16 feats
                for c, ch in enumerate(chunks):
                    its = []
                    for fc in range(FC):
                        itl = loop.tile([128, 128], bf16, tag="inT", name="inT",
                                        bufs=10)
                        nc.sync.dma_start(
                            itl[:], inputT_in[fc, :, ch * 128:(ch + 1) * 128])
                        its.append(itl)
                    ctype = chunk_type(ch)
                    # s-chunk: cols c*8..c*8+8 (and 40..48 for ch39's dis)
                    if ctype in ("rna", "dis"):
                        for fc in range(FC):
                            nc.tensor.matmul(pss[:, c * 8:(c + 1) * 8], its[fc][:],
                                             q_rhs[(ctype, fc)][:],
                                             start=(fc == 0), stop=(fc == FC - 1))
                    else:
                        for fc in range(FC):
                            nc.tensor.matmul(pss[:, c * 8:(c + 1) * 8], its[fc][:],
                                             q_rhs[("rna", fc)][:],
                                             start=(fc == 0), stop=(fc == FC - 1))
                        for fc in range(FC):
                            nc.tensor.matmul(pss[:, 40:48], its[fc][:],
                                             q_rhs[("dis", fc)][:],
                                             start=(fc == 0), stop=(fc == FC - 1))
                    # feats chunk: one FD=512 matmul per fc
                    psf = ps_feats.tile([128, 512], f32, tag="psf", name=f"psf{ch}")
                    if ctype in ("rna", "dis"):
                        for fc in range(FC):
                            nc.tensor.matmul(psf[:], its[fc][:],
                                             projb_all[(ctype, fc)][:],
                                             start=(fc == 0), stop=(fc == FC - 1))
                        fs = loop.tile([128, 512], bf16, tag="fsb", name="fsb",
                                       bufs=10)
                        nc.vector.tensor_copy(fs[:], psf[:])
                    else:
                        psf2 = ps_feats.tile([128, 512], f32, tag="psf",
                                             name=f"psf2{ch}")
                        for fc in range(FC):
                            nc.tensor.matmul(psf[:], its[fc][:],
                                             projb_all[("rna", fc)][:],
                                             start=(fc == 0), stop=(fc == FC - 1))
                            nc.tensor.matmul(psf2[:], its[fc][:],
                                             projb_all[("dis", fc)][:],
                                             start=(fc == 0), stop=(fc == FC - 1))
                        fs = loop.tile([128, 512], bf16, tag="fsb", name="fsb",
                                       bufs=10)
                        t1b = loop.tile([128, 512], bf16, tag="blendf",
                                        name="blendf", bufs=2)
                        nc.vector.tensor_scalar_mul(t1b[:], psf2[:], invsel39[:])
                        nc.vector.scalar_tensor_tensor(
                            fs[:], psf[:], sel39[:], t1b[:],
                            op0=Alu.mult, op1=Alu.add)
                    fsb.append(fs)

                # s columns to SBUF (+ ch39 blend), t5 = 0.2*t
                scol = loop.tile([128, 48], f32, tag="scol", name=f"scol{hs}",
                                 bufs=3)
                nc.vector.tensor_copy(scol[:], pss[:])
                if SPLIT_CH in chunks:
                    c39 = chunks.index(SPLIT_CH)
                    tb = loop.tile([128, 8], f32, tag="blends", name="blends",
                                   bufs=2)
                    nc.vector.tensor_scalar_mul(tb[:], scol[:, 40:48], invsel39[:])
                    tr = loop.tile([128, 8], f32, tag="blendr", name="blendr",
                                   bufs=2)
                    nc.vector.tensor_copy(tr[:], scol[:, c39 * 8:(c39 + 1) * 8])
                    nc.vector.scalar_tensor_tensor(
                        scol[:, c39 * 8:(c39 + 1) * 8],
                        tr[:], sel39[:], tb[:],
                        op0=Alu.mult, op1=Alu.add)
                t5 = loop.tile([128, 16], f32, tag="t5", name=f"t5{hs}", bufs=3)
                for c in range(CPH):
                    nc.vector.tensor_scalar_mul(
                        t5[:, c * 4:(c + 1) * 4],
                        scol[:, c * 8 + 4:c * 8 + 8], SLOPE)

                # -- elementwise: 16 tiles; prev half-stripe's bmm emitted
                #    between chunk 2 and chunk 3 so its AllGather latency
                #    hides under this half-stripe's elementwise work.
                last_hs = hs == N_HS - 1
                e_tiles = {}
                # last half-stripe splits d into two gathers to shorten the
                # serial tail; others use one 16-col gather.
                groups = [(0, 2), (2, 2)] if last_hs else [(0, 4)]
                dtiles = {}
                for g0, gn in groups:
                    dtiles[g0] = loop.tile([128, 4 * gn], f32, tag=f"dcol{gn}",
                                           name=f"dcol{hs}_{g0}", bufs=3)

                def grp(c):
                    for g0, gn in groups:
                        if g0 <= c < g0 + gn:
                            return g0, gn
                    raise AssertionError

                def elementwise(c, ch):
                    mT = mTs[c]
                    g0, _ = grp(c)
                    dtile = dtiles[g0]
                    for h in range(H):
                        tcol = scol[:, c * 8 + 4 + h:c * 8 + 4 + h + 1]
                        t5col = t5[:, c * 4 + h:c * 4 + h + 1]
                        zm = loop.tile([128, MY_N], bf16, tag="zm", name="zm",
                                       bufs=6)
                        nc.vector.tensor_add(zm[:], mT[:], S_b[h][:])
                        y = loop.tile([128, MY_N], bf16, tag="y", name="y", bufs=6)
                        if (c * H + h) % 16 < 8:
                            nc.scalar.activation(y[:], zm[:], Act.Prelu,
                                                 bias=tcol, scale=1.0, alpha=SLOPE)
                        else:
                            q1 = loop.tile([128, MY_N], bf16, tag="q1", name="q1",
                                           bufs=4)
                            nc.vector.tensor_scalar_add(q1[:], zm[:], tcol)
                            q2 = loop.tile([128, MY_N], bf16, tag="q2", name="q2",
                                           bufs=4)
                            nc.vector.tensor_scalar(q2[:], zm[:], SLOPE, t5col,
                                                    op0=Alu.mult, op1=Alu.add)
                            nc.vector.tensor_max(y[:], q1[:], q2[:])
                        e = epool.tile([128, MY_N], bf16, tag="e", name="e")
                        cl = (c - g0) * 4 + h
                        nc.scalar.activation(e[:], y[:], Act.Exp,
                                             accum_out=dtile[:, cl:cl + 1])
                        e_tiles[(c, h)] = e

                def issue_ag(key, dtile, din_t, dout_t):
                    nc.sync.dma_start(din_t[:], dtile[:])
                    nc.gpsimd.collective_compute(
                        "AllGather", Alu.bypass, replica_groups=RG,
                        ins=[din_t.opt()], outs=[dout_t.opt()])

                def emit_bmm(st):
                    p_e, p_fsb, key, dout_t, c0, ncc, p_last = st
                    ncols = 4 * ncc
                    # complete d only now: the AllGather latency hid under
                    # the elementwise work since its issue.
                    dg = loop.tile([128, 16, N_CORES], f32, tag="dg",
                                   name=f"dg{key}", bufs=3)
                    for r in range(N_CORES):
                        nc.sync.dma_start(dg[:, 0:ncols, r],
                                          dout_t[r * 128:(r + 1) * 128, :])
                    d_sum = loop.tile([128, 16], f32, tag="dsum",
                                      name=f"dsum{key}", bufs=3)
                    nc.vector.tensor_reduce(d_sum[:, 0:ncols],
                                            dg[:, 0:ncols, :],
                                            mybir.AxisListType.X, Alu.add)
                    p_dinv = loop.tile([128, 16], f32, tag="dinv",
                                       name=f"dinv{key}", bufs=3)
                    nc.vector.reciprocal(p_dinv[:, 0:ncols], d_sum[:, 0:ncols])
                    for c in range(c0, c0 + ncc):
                        for h in range(H):
                            g4 = loop.tile([128, 128], bf16, tag="g4", name="g4",
                                           bufs=6)
                            nc.vector.tensor_scalar_mul(
                                g4[:], p_fsb[c][:, h * 128:(h + 1) * 128],
                                p_dinv[:, (c - c0) * 4 + h:(c - c0) * 4 + h + 1])
                            e = p_e[(c, h)]
                            last = p_last and c == c0 + ncc - 1 and h == H - 1
                            nc.tensor.matmul(vals_ps[:, 0:512], g4[:],
                                             e[:, 0:512],
                                             start=first_mm[0], stop=last)
                            nc.tensor.matmul(vals_ps[:, 512:1024], g4[:],
                                             e[:, 512:1024],
                                             start=first_mm[0], stop=last)
                            first_mm[0] = False

                for c, ch in list(enumerate(chunks))[:2]:
                    elementwise(c, ch)
                if last_hs:
                    issue_ag("15a", dtiles[0], d15_in[0], d15_out[0])
                elementwise(2, chunks[2])
                for st in pendings:
                    emit_bmm(st)
                pendings = []
                elementwise(3, chunks[3])

                if last_hs:
                    issue_ag("15b", dtiles[2], d15_in[1], d15_out[1])
                    pendings = [
                        (e_tiles, fsb, "15a", d15_out[0], 0, 2, False),
                        (e_tiles, fsb, "15b", d15_out[1], 2, 2, True),
                    ]
                else:
                    issue_ag(hs, dtiles[0], d_in[hs], d_out[hs])
                    pendings = [(e_tiles, fsb, hs, d_out[hs], 0, 4, False)]

            for st in pendings:
                emit_bmm(st)

            # ---- tail: instance norm + residual + elu ------------------
            epool.release()
            loop.release()
            tailp = tc.alloc_tile_pool(name="tail", bufs=1)
            vs = tailp.tile([128, MY_N], f32, tag="vs", name="vs")
            srow1 = tailp.tile([128, 1], f32, tag="srow1", name="srow1")
            nc.scalar.activation(vs[:], vals_ps[:], Act.Copy, scale=0.25,
                                 accum_out=srow1[:])
            vsq = tailp.tile([128, MY_N], f32, tag="vsq", name="vsq")
            srow2 = tailp.tile([128, 1], f32, tag="srow2", name="srow2")
            nc.scalar.activation(vsq[:], vs[:], Act.Square, accum_out=srow2[:])

            ps1 = ps_s.tile([128, 16], f32, tag="pss", name="ps1")
            nc.tensor.matmul(ps1[0:1, 0:1], srow1[:], ones_col[:])
            ps2 = ps_s.tile([128, 16], f32, tag="pss", name="ps2")
            nc.tensor.matmul(ps2[0:1, 0:1], srow2[:], ones_col[:])
            stv = tailp.tile([1, 32], f32, tag="stv", name="stv")
            nc.vector.memset(stv[:], 0.0)
            nc.vector.tensor_copy(stv[0:1, 0:1], ps1[0:1, 0:1])
            nc.vector.tensor_copy(stv[0:1, 16:17], ps2[0:1, 0:1])
            nc.sync.dma_start(st_in[:], stv[:])
            nc.gpsimd.collective_compute(
                "AllReduce", Alu.add, replica_groups=RG,
                ins=[st_in.opt()], outs=[st_out.opt()])
            str_ = tailp.tile([1, 32], f32, tag="str", name="str")
            nc.sync.dma_start(str_[:], st_out[:])

            c = 1.0 / float(N * O)
            mu = tailp.tile([1, 1], f32, tag="mu", name="mu")
            nc.vector.tensor_scalar_mul(mu[:], str_[0:1, 0:1], c)
            m2 = tailp.tile([1, 1], f32, tag="m2", name="m2")
            nc.vector.tensor_scalar_mul(m2[:], str_[0:1, 16:17], c)
            mu2 = tailp.tile([1, 1], f32, tag="mu2", name="mu2")
            nc.vector.tensor_mul(mu2[:], mu[:], mu[:])
            var = tailp.tile([1, 1], f32, tag="var", name="var")
            nc.vector.tensor_sub(var[:], m2[:], mu2[:])
            vpe = tailp.tile([1, 1], f32, tag="vpe", name="vpe")
            nc.vector.tensor_scalar_add(vpe[:], var[:], EPS)
            sd = tailp.tile([1, 1], f32, tag="sd", name="sd")
            nc.scalar.activation(sd[:], vpe[:], Act.Sqrt)
            rstd = tailp.tile([1, 1], f32, tag="rstd", name="rstd")
            nc.vector.reciprocal(rstd[:], sd[:])
            negmurs = tailp.tile([1, 1], f32, tag="negmurs", name="negmurs")
            nc.vector.tensor_mul(negmurs[:], mu[:], rstd[:])
            nc.vector.tensor_scalar_mul(negmurs[:], negmurs[:], -1.0)

            a_col = tailp.tile([128, 1], f32, tag="acol", name="acol")
            nc.gpsimd.partition_broadcast(a_col[:], rstd[:])
            b_row = tailp.tile([1, 128], f32, tag="brow", name="brow")
            nc.scalar.activation(b_row[:], ones_row[0:1, 0:128], Act.Copy,
                                 scale=negmurs[:])

            r_ps = ps_vals.tile([128, MY_N], f32, tag="big", name="resid")
            for half in range(2):
                sl = slice(half * 512, (half + 1) * 512)
                for fc in range(FC):
                    nc.tensor.matmul(r_ps[:, sl], wrT[fc][:], rowsT[fc][:, sl],
                                     start=(fc == 0), stop=False)
                nc.tensor.matmul(r_ps[:, sl], b_row[:], ones_row[:],
                                 start=False, stop=True)

            pre = tailp.tile([128, MY_N], f32, tag="pre", name="pre")
            nc.vector.scalar_tensor_tensor(pre[:], vs[:], a_col[:], r_ps[:],
                                           op0=Alu.mult, op1=Alu.add)
            negp = tailp.tile([128, MY_N], f32, tag="negp", name="negp")
            nc.vector.tensor_scalar_min(negp[:], pre[:], 0.0)
            w = tailp.tile([128, MY_N], f32, tag="w", name="w")
            nc.scalar.activation(w[:], negp[:], Act.Exp)
            r1 = tailp.tile([128, MY_N], f32, tag="r1", name="r1")
            nc.vector.tensor_scalar_max(r1[:], pre[:], 0.0)
            outt = tailp.tile([128, MY_N], f32, tag="outt", name="outt")
            nc.vector.scalar_tensor_tensor(outt[:], w[:], -1.0, r1[:],
                                           op0=Alu.add, op1=Alu.add)
            nc.sync.dma_start(out_dram, outt[:])
            tailp.release()

    nc.compile()
    return nc


def _get_nc():
    if "nc" not in _cached:
        _cached["nc"] = _build()
    return _cached["nc"]


def kernel(input_mat, connectivity_mask, proj_rna, proj_dis, score_src,
           score_tgt, residual_w):
    import ml_dtypes
    from concourse.bass_utils import run_bass_kernel_spmd

    bf = ml_dtypes.bfloat16
    nc = _get_nc()
    input_mat = np.asarray(input_mat, np.float32)
    connectivity_mask = np.asarray(connectivity_mask, np.float32)
    ident = np.eye(128, dtype=np.float32)
    sel39 = (np.arange(128) < SPLIT_ROW).astype(np.float32)[:, None]
    rna_mask = (np.arange(N) < N_RNA).astype(np.float32)[:, None]

    inputT = np.ascontiguousarray(input_mat.T).astype(bf)      # [F, N]
    residual_wT_np = np.ascontiguousarray(
        np.asarray(residual_w, np.float32).T).astype(bf)       # [F, O]

    in_maps = []
    for k in range(N_CORES):
        r0, r1 = k * MY_N, (k + 1) * MY_N
        maskT_k = np.ascontiguousarray(
            connectivity_mask[r0:r1, :].T).astype(bf)          # [N, MY_N]
        in_maps.append({
            "maskT": maskT_k,
            "inputT": inputT.reshape(FC, 128, N),
            "inputT_my": np.ascontiguousarray(inputT[:, r0:r1]).reshape(
                FC, 128, MY_N),
            "rowmask": rna_mask[r0:r1, 0].astype(bf)[None, :],
            "proj_rna": np.asarray(proj_rna, np.float32),
            "proj_dis": np.asarray(proj_dis, np.float32),
            "score_src": np.asarray(score_src, np.float32),
            "score_tgt": np.asarray(score_tgt, np.float32),
            "residual_wT": residual_wT_np.reshape(FC, 128, O),
            "identf": ident,
            "sel39": sel39,
            "invsel39": 1.0 - sel39,
        })

    res = run_bass_kernel_spmd(nc, in_maps, core_ids=list(range(N_CORES)))
    _cached["last_result"] = res
    out = np.empty((N, O), np.float32)
    for k in range(N_CORES):
        out[k * MY_N:(k + 1) * MY_N, :] = res.results[k]["out"].T
    return out
